# revision 1
# baseline (speedup 1.0000x reference)
"""Trainium2 Bass kernel for nn_Block_50706383897045 (dense transformer block).

Strategy: data-parallel over batch — B=8 equals n_cores=8, one batch element
per core, no collectives. Per core the full block (LN -> QKV -> causal
attention -> out-proj -> residual -> LN -> MLP(gelu) -> residual) runs on a
[T=1024, C=768] slice.

Layout: activations live feature-major in SBUF ([128-feature partitions x
tokens free]) so every GEMM contracts over the partition dim with zero
transposes between stages. Attention computes scores transposed
(scoresT[k_tok, q_tok]) so the attn@v contraction consumes exp(scores)
directly; the softmax denominator comes free from a ones-column appended to
the V tile; normalization is deferred to after attn@v (linearity). No
max-subtraction is needed: |scores| < ~70 so exp stays in fp32 range.
Matmuls run in float32r (~1.5e-4 rel err, 4x faster than fp32).
"""
import sys

sys.path.insert(0, "/opt/trn_rl_repo")

import numpy as np

import concourse.bass as bass
import concourse.bacc as bacc
import concourse.mybir as mybir
import concourse.tile as tile
from concourse import bass_utils
from concourse.masks import make_identity

AF = mybir.ActivationFunctionType
ALU = mybir.AluOpType
f32 = mybir.dt.float32
f32r = mybir.dt.float32r

B, T, C, H, D = 8, 1024, 768, 12, 64
F = C // 128      # 6 feature tiles of the residual stream
NT = T // 128     # 8 token tiles
CH = 512          # token chunk (fp32 moving-operand max)
NCH = T // CH     # 2
M3 = 4 * C        # 3072 MLP hidden
MTH = 12          # MLP mid tiles per half
EPS = 1e-5

_NC_CACHE = None
ABLATE = set()  # {'attn_core','mlp_mm','exp_off','masks_off','bcast_off'}


def _chunk(c):
    return slice(c * CH, (c + 1) * CH)


def _ln_stats(nc, stats_ps, sq_pool, ones_col, src, c, tag):
    """Accumulate sum and sum-of-squares over the 6 partition tiles of
    chunk c: two [1, CH] PSUM rows via ones-column matmuls."""
    sl = _chunk(c)
    ps_sum = stats_ps.tile([1, CH], f32, tag="lnsum", name=f"{tag}_sum{c}")
    ps_sq = stats_ps.tile([1, CH], f32, tag="lnsq", name=f"{tag}_sq{c}")
    for f in range(F):
        sq_t = sq_pool.tile([128, CH], f32r, tag="ln_sq", name="sq_t")
        nc.scalar.activation(sq_t, src[:, f, sl], AF.Square)
        nc.tensor.matmul(ps_sum, ones_col, src[:, f, sl],
                         start=(f == 0), stop=(f == F - 1))
        nc.tensor.matmul(ps_sq, ones_col, sq_t,
                         start=(f == 0), stop=(f == F - 1))
    return ps_sum, ps_sq


def _ln_finish(nc, pools, stats, src, dst, wcol, bcol, eps_t, c):
    """mean/var -> rstd on [1, CH]; broadcast over partitions (gpsimd);
    normalize chunk c of src into dst with ln_w/ln_b columns."""
    row_pool, bc_pool, tmp_pool = pools
    ps_sum, ps_sq = stats
    sl = _chunk(c)
    mean = row_pool.tile([1, CH], f32, tag="ln_ra", name="mean")
    nc.vector.tensor_scalar_mul(mean, ps_sum, 1.0 / C)
    m2 = row_pool.tile([1, CH], f32, tag="ln_rb", name="m2")
    nc.vector.tensor_scalar_mul(m2, ps_sq, 1.0 / C)
    msq = row_pool.tile([1, CH], f32, tag="ln_rc", name="msq")
    nc.vector.tensor_mul(msq, mean, mean)
    nc.vector.tensor_sub(m2, m2, msq)          # m2 <- var
    nc.scalar.activation(msq, m2, AF.Sqrt, bias=eps_t)   # msq <- std
    rstd = row_pool.tile([1, CH], f32, tag="ln_rb", name="rstd")
    nc.vector.reciprocal(rstd, msq)
    negmu = mean
    nc.vector.tensor_scalar_mul(negmu, mean, -1.0)
    bc_mu = bc_pool.tile([128, CH], f32, tag="ln_bcmu", name="bc_mu")
    nc.gpsimd.partition_broadcast(bc_mu, negmu)
    bc_rs = bc_pool.tile([128, CH], f32, tag="ln_bcrs", name="bc_rs")
    nc.gpsimd.partition_broadcast(bc_rs, rstd)
    for f in range(F):
        tmp = tmp_pool.tile([128, CH], f32, tag="ln_tmp", name="tmp")
        nc.vector.tensor_add(tmp, src[:, f, sl], bc_mu)
        nc.vector.tensor_mul(tmp, tmp, bc_rs)
        nc.scalar.activation(
            dst[:, f, sl], tmp, AF.Identity,
            bias=bcol[:, f:f + 1], scale=wcol[:, f:f + 1])


def _build(chain=1):
    nc = bacc.Bacc("TRN2", target_bir_lowering=False, debug=False,
                   num_devices=8)

    x_d = nc.dram_tensor("x", [T, C], f32, kind="ExternalInput")
    wqkv_d = nc.dram_tensor("w_qkv", [C, 3 * C], f32, kind="ExternalInput")
    bqkv_d = nc.dram_tensor("b_qkv", [3 * C], f32, kind="ExternalInput")
    wout_d = nc.dram_tensor("w_out", [C, C], f32, kind="ExternalInput")
    bout_d = nc.dram_tensor("b_out", [C], f32, kind="ExternalInput")
    wc1_d = nc.dram_tensor("w_c1", [C, M3], f32, kind="ExternalInput")
    bc1_d = nc.dram_tensor("b_c1", [M3], f32, kind="ExternalInput")
    wc2_d = nc.dram_tensor("w_c2", [M3, C], f32, kind="ExternalInput")
    bc2_d = nc.dram_tensor("b_c2", [C], f32, kind="ExternalInput")
    lnw_d = nc.dram_tensor("ln_w", [C], f32, kind="ExternalInput")
    lnb_d = nc.dram_tensor("ln_b", [C], f32, kind="ExternalInput")
    y_d = nc.dram_tensor("y", [T, C], f32, kind="ExternalOutput")
    mids = [nc.dram_tensor(f"mid{i}", [T, C], f32)
            for i in range(chain - 1)]

    with tile.TileContext(nc) as tc:
        srcs = [x_d] + mids
        dsts = mids + [y_d]
        for r in range(chain):
            _kernel_body(nc, tc, srcs[r], wqkv_d, bqkv_d, wout_d, bout_d,
                         wc1_d, bc1_c_d := bc1_d, wc2_d, bc2_d, lnw_d,
                         lnb_d, dsts[r])
    nc.compile()
    return nc


def _kernel_body(nc, tc, x_d, wqkv_d, bqkv_d, wout_d, bout_d,
                 wc1_d, bc1_d, wc2_d, bc2_d, lnw_d, lnb_d, y_d):
    with tc.tile_pool(name="persist", bufs=1) as persist:
        # ---- constants / small params ----
        ident = persist.tile([128, 128], f32)
        make_identity(nc, ident)
        ones_col = persist.tile([128, 1], f32r)
        nc.vector.memset(ones_col.bitcast(f32), 1.0)
        eps_t = persist.tile([1, 1], f32)
        nc.vector.memset(eps_t, EPS)
        lnw_c = persist.tile([128, F], f32)
        nc.sync.dma_start(lnw_c, lnw_d.ap().rearrange("(o p) -> p o", p=128))
        lnb_c = persist.tile([128, F], f32)
        nc.sync.dma_start(lnb_c, lnb_d.ap().rearrange("(o p) -> p o", p=128))
        bqkv_c = persist.tile([128, 12], f32)  # k,q bias columns only
        nc.sync.dma_start(
            bqkv_c, bqkv_d.ap()[0:2 * C].rearrange("(o p) -> p o", p=128))
        bout_c = persist.tile([128, F], f32)
        nc.sync.dma_start(bout_c, bout_d.ap().rearrange("(o p) -> p o", p=128))
        bc1_c = persist.tile([128, 24], f32)
        nc.sync.dma_start(bc1_c, bc1_d.ap().rearrange("(o p) -> p o", p=128))
        bc2_c = persist.tile([128, F], f32)
        nc.sync.dma_start(bc2_c, bc2_d.ap().rearrange("(o p) -> p o", p=128))
        # v-part bias broadcast along partitions: [128, 768] (free-dim bias
        # for the token-major V tiles)
        # 0/1 causal mask tiles for the 4 diagonal offsets; applied on DVE
        # for half the diagonal tiles to balance DVE vs gpsimd load
        masks = persist.tile([128, 4, CH], f32)
        nc.vector.memset(masks, 1.0)
        for mi in range(4):
            nc.gpsimd.affine_select(
                out=masks[:, mi, :], in_=masks[:, mi, :],
                compare_op=ALU.is_ge, fill=0.0,
                base=-mi * 128, pattern=[[1, CH]], channel_multiplier=-1)
        bv_bc = persist.tile([128, C], f32)
        bv_src = bqkv_d.ap()[2 * C:3 * C]
        bv_b = bass.AP(tensor=bv_src.tensor, offset=bv_src.offset,
                       ap=[[0, 128]] + [list(p) for p in bv_src.ap])
        nc.gpsimd.dma_start(out=bv_bc, in_=bv_b)

        with (
            tc.tile_pool(name="resid", bufs=1) as resid_pool,
            tc.tile_pool(name="attnout", bufs=1) as ao_pool,
            tc.tile_pool(name="hpool", bufs=1) as h_pool,
        ):
            x_fm = resid_pool.tile([128, F, T], f32r, tag="x_slot",
                                   name="x_fm")
            attn_out = ao_pool.tile([128, F, T], f32r, tag="attn_out",
                                    name="attn_out")
            h_fm = h_pool.tile([128, F, T], f32r, tag="h_slot", name="h_fm")

            with (
                tc.tile_pool(name="wv", bufs=1) as wv_pool,
                tc.tile_pool(name="wkq", bufs=1) as wkq_pool,
                tc.tile_pool(name="v1pool", bufs=1) as v1_pool,
            ):
                with tc.tile_pool(name="xtm", bufs=1) as xtm_pool:
                    # input x first (unblocks the transpose pipeline), then
                    # qkv weights (row tiles, contiguous)
                    xtm_t = []
                    for t in range(NT):
                        x_tm = xtm_pool.tile([128, C], f32, tag=f"x_tm{t % 4}",
                                             name=f"x_tm{t}")
                        if "xsplit" in ABLATE:
                            for f in range(F):
                                fs = slice(f * 128, (f + 1) * 128)
                                nc.sync.dma_start(
                                    x_tm[:, fs],
                                    x_d.ap()[t * 128:(t + 1) * 128, fs])
                        else:
                            nc.sync.dma_start(
                                x_tm, x_d.ap()[t * 128:(t + 1) * 128, :])
                        xtm_t.append(x_tm)
                    wv_t, wkq_t = [], []
                    for kt in range(F):
                        wt = wv_pool.tile([128, C], f32r, tag=f"wv{kt}",
                                          name=f"wv{kt}")
                        nc.sync.dma_start(
                            wt, wqkv_d.ap().bitcast(f32r)
                            [kt * 128:(kt + 1) * 128, 2 * C:3 * C])
                        wv_t.append(wt)
                    for kt in range(F):
                        wt = wkq_pool.tile([128, 2 * C], f32r,
                                           tag=f"wkq{kt}", name=f"wkq{kt}")
                        nc.sync.dma_start(
                            wt, wqkv_d.ap().bitcast(f32r)
                            [kt * 128:(kt + 1) * 128, 0:2 * C])
                        wkq_t.append(wt)

                    _phase_a(nc, tc, xtm_t, x_fm, h_fm, ident, ones_col,
                             lnw_c, lnb_c, eps_t)
                _qkv_attention(nc, tc, v1_pool, wv_t, wkq_t, h_fm,
                               attn_out, bqkv_c, bv_bc, masks)

            with (
                tc.tile_pool(name="x2pool", bufs=1) as x2_pool,
                tc.tile_pool(name="wc1a", bufs=1) as wc1a_pool,
            ):
                x2_fm = x2_pool.tile([128, F, T], f32r, tag="x2",
                                     name="x2_fm")
                h2_fm, wc1a_t = _outproj_ln2(
                    nc, tc, h_pool, wc1a_pool, x_fm, x2_fm, attn_out,
                    wout_d, wc1_d, bout_c, lnw_c, lnb_c, eps_t, ones_col)
                out_fm = resid_pool.tile([128, F, T], f32, tag="x_slot",
                                         name="out_fm")
                _mlp(nc, tc, wc1a_t, h2_fm, x2_fm, out_fm, wc1_d, wc2_d,
                     bc1_c, bc2_c, ident, y_d)


def _phase_a(nc, tc, xtm_t, x_fm, h_fm, ident, ones_col, lnw_c, lnb_c,
             eps_t):
    """Transpose x to feature-major, LN1 (stats interleaved with the
    transposes so PE stays busy)."""
    with (
        tc.tile_pool(name="trps", bufs=4, space="PSUM") as tr_ps,
        tc.tile_pool(name="lnps", bufs=2, space="PSUM") as stats_ps,
        tc.tile_pool(name="ln1_rows", bufs=1) as row_pool,
        tc.tile_pool(name="ln1_bc", bufs=1) as bc_pool,
        tc.tile_pool(name="ln1_tmp", bufs=2) as tmp_pool,
        tc.tile_pool(name="ln1_sq", bufs=2) as sq_pool,
    ):
        with nc.named_scope("load_ln1"):
            stats = []
            for c in range(NCH):
                for t in range(4 * c, 4 * (c + 1)):
                    x_tm = xtm_t[t]
                    for f in range(F):
                        ps = tr_ps.tile([128, 128], f32, tag="tr",
                                        name="tr")
                        nc.tensor.transpose(
                            ps, x_tm[:, f * 128:(f + 1) * 128], ident)
                        nc.vector.tensor_copy(
                            x_fm[:, f, t * 128:(t + 1) * 128], ps)
                stats.append(_ln_stats(nc, stats_ps, sq_pool, ones_col,
                                       x_fm, c, "ln1"))
            for c in range(NCH):
                _ln_finish(nc, (row_pool, bc_pool, tmp_pool), stats[c],
                           x_fm, h_fm, lnw_c, lnb_c, eps_t, c)


def _qkv_attention(nc, tc, v1_pool, wv_t, wkq_t, h_fm, attn_out,
                   bqkv_c, bv_bc, masks):
    # V with appended ones column per head (softmax denominator)
    v1 = v1_pool.tile([128, NT, H * 65], f32r, tag="v1", name="v1")
    nc.vector.memset(
        v1.rearrange("p t (h m) -> p t h m", m=65)[:, :, :, 64:65]
        .bitcast(f32), 1.0)
    with tc.tile_pool(name="vps", bufs=3, space="PSUM") as v_ps:
        with nc.named_scope("qkv_v"):
            for t in range(NT):
                for half in range(2):
                    ps_v = v_ps.tile([128, 384], f32, tag="vps",
                                     name="ps_v")
                    c0 = half * 384
                    for kt in range(F):
                        nc.tensor.matmul(
                            ps_v, h_fm[:, kt, t * 128:(t + 1) * 128],
                            wv_t[kt][:, c0:c0 + 384],
                            start=(kt == 0), stop=(kt == F - 1))
                    dst = (v1[:, t, :].rearrange("p (h m) -> p h m", m=65)
                           [:, half * 6:(half + 1) * 6, 0:64])
                    src = ps_v.rearrange("p (h m) -> p h m", m=64)
                    bias = (bv_bc[:, half * 384:(half + 1) * 384]
                            .rearrange("p (h m) -> p h m", m=64))
                    nc.vector.tensor_add(dst, src, bias)

    # K,Q per feature tile f (heads 2f, 2f+1), then attention
    with (
        tc.tile_pool(name="kqt", bufs=2) as kq_pool,
        tc.tile_pool(name="kqps", bufs=2, space="PSUM") as kq_ps,
        tc.tile_pool(name="sps", bufs=3, space="PSUM") as s_ps,
        tc.tile_pool(name="yps", bufs=2, space="PSUM") as y_ps,
        tc.tile_pool(name="expp", bufs=4) as exp_pool,
        tc.tile_pool(name="attn_sm", bufs=2) as asm_pool,
        tc.tile_pool(name="attn_bcp", bufs=2) as abc_pool,
    ):
        for f in range(F):
            with nc.named_scope(f"kq_{f}"):
                kf = kq_pool.tile([128, T], f32r, tag="kf")
                qf = kq_pool.tile([128, T], f32r, tag="qf")
                for dst_t, col0, bcol in (
                        (kf, f * 128, f), (qf, C + f * 128, F + f)):
                    for c in range(NCH):
                        sl = _chunk(c)
                        ps = kq_ps.tile([128, CH], f32, tag="kqps",
                                        name="kq_ps")
                        for kt in range(F):
                            nc.tensor.matmul(
                                ps, wkq_t[kt][:, col0:col0 + 128],
                                h_fm[:, kt, sl],
                                start=(kt == 0), stop=(kt == F - 1))
                        nc.vector.tensor_scalar_add(
                            dst_t[:, sl], ps, bqkv_c[:, bcol:bcol + 1])
            for hl in range(2):
                h_idx = 2 * f + hl
                base = hl * 64
                with nc.named_scope(f"attn_h{h_idx}"):
                    if "attn_core" in ABLATE:
                        continue
                    for c in range(NCH):
                        sl = _chunk(c)
                        kt_max = 4 * (c + 1)
                        ps_y = y_ps.tile([65, CH], f32, tag="yps",
                                         name="ps_y")
                        for kt in range(kt_max):
                            ps_s = s_ps.tile([128, CH], f32, tag="sps",
                                             name="ps_s")
                            nc.tensor.matmul(
                                ps_s,
                                kf[base:base + 64,
                                   kt * 128:(kt + 1) * 128],
                                qf[base:base + 64, sl],
                                start=True, stop=True)
                            expt = exp_pool.tile([128, CH], f32r,
                                                 tag="expt", name="expt")
                            if "exp_off" in ABLATE:
                                nc.vector.tensor_copy(expt, ps_s)
                            else:
                                nc.scalar.activation(expt, ps_s, AF.Exp)
                            off = c * CH - kt * 128
                            if off < 128 and "masks_off" not in ABLATE:
                                # diagonal tile: causal mask (alternate
                                # engines to balance DVE/gpsimd load)
                                if (kt + h_idx) % 2 == 0:
                                    nc.gpsimd.affine_select(
                                        out=expt, in_=expt,
                                        compare_op=ALU.is_ge, fill=0.0,
                                        base=off, pattern=[[1, CH]],
                                        channel_multiplier=-1)
                                else:
                                    nc.vector.tensor_mul(
                                        expt, expt,
                                        masks[:, (-off) // 128, :])
                            nc.tensor.matmul(
                                ps_y,
                                v1[:, kt, h_idx * 65:h_idx * 65 + 65],
                                expt,
                                start=(kt == 0), stop=(kt == kt_max - 1))
                        recip = asm_pool.tile([1, CH], f32, tag="recip",
                                              name="recip")
                        nc.vector.reciprocal(recip, ps_y[64:65, :])
                        bc = abc_pool.tile([64, CH], f32, tag="attn_bc",
                                           name="bc")
                        if "bcast_off" in ABLATE:
                            nc.vector.tensor_copy(
                                attn_out[base:base + 64, f, sl],
                                ps_y[0:64, :])
                        else:
                            nc.gpsimd.partition_broadcast(bc, recip)
                            nc.vector.tensor_mul(
                                attn_out[base:base + 64, f, sl],
                                ps_y[0:64, :], bc)


def _outproj_ln2(nc, tc, h_pool, wc1a_pool, x_fm, x2_fm, attn_out, wout_d,
                 wc1_d, bout_c, lnw_c, lnb_c, eps_t, ones_col):
    with (
        tc.tile_pool(name="ln2ps", bufs=2, space="PSUM") as stats_ps2,
        tc.tile_pool(name="ln2_rows", bufs=1) as row_pool2,
        tc.tile_pool(name="ln2_bc", bufs=2) as bc_pool2,
        tc.tile_pool(name="ln2_tmp", bufs=2) as tmp_pool2,
        tc.tile_pool(name="ln2_sq", bufs=2) as sq_pool2,
    ):
        ln2_stats = []
        with (
            tc.tile_pool(name="woutp", bufs=1) as wout_pool,
            tc.tile_pool(name="ops", bufs=3, space="PSUM") as o_ps,
            tc.tile_pool(name="otmp", bufs=2) as otmp_pool,
        ):
            wout_t = []
            for kt in range(F):
                wt = wout_pool.tile([128, C], f32r, tag=f"wout{kt}",
                                    name=f"wout{kt}")
                nc.sync.dma_start(
                    wt, wout_d.ap().bitcast(f32r)
                    [kt * 128:(kt + 1) * 128, :])
                wout_t.append(wt)
            # prefetch first-half MLP weights (after wout in queue order)
            wc1a_t = []
            for kt in range(F):
                wt = wc1a_pool.tile([128, M3 // 2], f32r,
                                    tag=f"wc1a{kt}", name=f"wc1a{kt}")
                nc.sync.dma_start(
                    wt, wc1_d.ap().bitcast(f32r)
                    [kt * 128:(kt + 1) * 128, 0:M3 // 2])
                wc1a_t.append(wt)
            with nc.named_scope("out_proj"):
                for c in range(NCH):
                    sl = _chunk(c)
                    for ct in range(F):
                        ps = o_ps.tile([128, CH], f32, tag="ops",
                                       name="o_ps")
                        for kt in range(F):
                            nc.tensor.matmul(
                                ps, wout_t[kt][:, ct * 128:(ct + 1) * 128],
                                attn_out[:, kt, sl],
                                start=(kt == 0), stop=(kt == F - 1))
                        tmp = otmp_pool.tile([128, CH], f32, tag="otmp",
                                             name="o_tmp")
                        nc.scalar.activation(
                            tmp, ps, AF.Identity, bias=bout_c[:, ct:ct + 1])
                        nc.vector.tensor_add(
                            x2_fm[:, ct, sl], tmp, x_fm[:, ct, sl])
                for c in range(NCH):
                    ln2_stats.append(_ln_stats(
                        nc, stats_ps2, sq_pool2, ones_col, x2_fm, c, "ln2"))

        h2_fm = h_pool.tile([128, F, T], f32r, tag="h_slot", name="h2_fm")
        with nc.named_scope("ln2"):
            for c in range(NCH):
                _ln_finish(nc, (row_pool2, bc_pool2, tmp_pool2),
                           ln2_stats[c], x2_fm, h2_fm, lnw_c, lnb_c,
                           eps_t, c)
    return h2_fm, wc1a_t


def _mlp(nc, tc, wc1a_t, h2_fm, x2_fm, out_fm, wc1_d, wc2_d, bc1_c, bc2_c,
         ident, y_d):
    with (
        tc.tile_pool(name="wc1b", bufs=1) as wc1b_pool,
        tc.tile_pool(name="wc2s", bufs=3) as wc2_pool,
        tc.tile_pool(name="mlpout", bufs=1, space="PSUM") as mo_ps,
        tc.tile_pool(name="mlpc1", bufs=2, space="PSUM") as c1_ps,
        tc.tile_pool(name="gp", bufs=3) as g_pool,
        tc.tile_pool(name="mtmp", bufs=2) as mtmp_pool,
        tc.tile_pool(name="otm", bufs=2) as otm_pool,
    ):
        # second-half c1 weights prefetch (runs during half 0)
        wc1b_t = []
        for kt in range(F):
            wt = wc1b_pool.tile([128, M3 // 2], f32r, tag=f"wc1b{kt}",
                                name=f"wc1b{kt}")
            nc.sync.dma_start(
                wt, wc1_d.ap().bitcast(f32r)
                [kt * 128:(kt + 1) * 128, M3 // 2:M3])
            wc1b_t.append(wt)
        for half in range(2):
            wc1_t = wc1a_t if half == 0 else wc1b_t
            with nc.named_scope(f"mlp_h{half}"):
                for c in range(NCH):
                    sl = _chunk(c)
                    wc2_t = []
                    for mt in range(MTH):
                        row0 = (half * MTH + mt) * 128
                        wt = wc2_pool.tile([128, C], f32r, tag="wc2",
                                           name=f"wc2_{mt}")
                        nc.sync.dma_start(
                            wt, wc2_d.ap().bitcast(f32r)
                            [row0:row0 + 128, :])
                        wc2_t.append(wt)
                    ps_out = [mo_ps.tile([128, CH], f32, tag=f"mo{ct}",
                                         name=f"mo{ct}")
                              for ct in range(F)]
                    g_prev = None
                    if "mlp_mm" in ABLATE:
                        for ct in range(F):
                            nc.tensor.matmul(
                                ps_out[ct],
                                wc2_t[0][:, ct * 128:(ct + 1) * 128],
                                h2_fm[:, 0, sl], start=True, stop=True)
                    for mt in range(MTH if "mlp_mm" not in ABLATE else 0):
                        ps_g = c1_ps.tile([128, CH], f32, tag="c1ps",
                                          name="ps_g")
                        for kt in range(F):
                            nc.tensor.matmul(
                                ps_g,
                                wc1_t[kt][:, mt * 128:(mt + 1) * 128],
                                h2_fm[:, kt, sl],
                                start=(kt == 0), stop=(kt == F - 1))
                        # pipelined: c2(mt-1) traced after c1(mt)
                        if mt > 0 and "c2_off" not in ABLATE:
                            for ct in range(F):
                                nc.tensor.matmul(
                                    ps_out[ct],
                                    wc2_t[mt - 1]
                                    [:, ct * 128:(ct + 1) * 128],
                                    g_prev,
                                    start=(mt == 1), stop=False)
                        g_t = g_pool.tile([128, CH], f32r, tag="g",
                                          name="g_t")
                        if "gelu_id" in ABLATE:
                            nc.scalar.activation(
                                g_t, ps_g, AF.Identity,
                                bias=bc1_c[:, half * MTH + mt:
                                           half * MTH + mt + 1])
                        elif "gelu_off" in ABLATE:
                            nc.vector.tensor_copy(g_t, ps_g)
                        else:
                            nc.scalar.activation(
                                g_t, ps_g, AF.Gelu,
                                bias=bc1_c[:, half * MTH + mt:
                                           half * MTH + mt + 1])
                        g_prev = g_t
                    if "mlp_mm" not in ABLATE:
                        for ct in range(F):
                            nc.tensor.matmul(
                                ps_out[ct],
                                wc2_t[MTH - 1][:, ct * 128:(ct + 1) * 128],
                                g_prev, start=("c2_off" in ABLATE),
                                stop=True)
                    for ct in range(F):
                        if half == 0:
                            nc.vector.tensor_add(
                                out_fm[:, ct, sl], ps_out[ct],
                                x2_fm[:, ct, sl])
                        else:
                            tmp = mtmp_pool.tile([128, CH], f32,
                                                 tag="mtmp", name="m_tmp")
                            nc.scalar.activation(
                                tmp, ps_out[ct], AF.Identity,
                                bias=bc2_c[:, ct:ct + 1])
                            nc.vector.tensor_add(
                                out_fm[:, ct, sl], out_fm[:, ct, sl], tmp)
                    if half == 1:
                        # store this chunk: transpose back to token-major
                        with nc.named_scope(f"store_c{c}"):
                            for t in range(4 * c, 4 * (c + 1)):
                                o_tm = otm_pool.tile([128, C], f32,
                                                     tag="o_tm",
                                                     name="o_tm")
                                for f in range(F):
                                    ps = mo_ps.tile([128, 128], f32,
                                                    tag=f"mo{f}",
                                                    name="otr")
                                    nc.tensor.transpose(
                                        ps,
                                        out_fm[:, f,
                                               t * 128:(t + 1) * 128],
                                        ident)
                                    nc.vector.tensor_copy(
                                        o_tm[:, f * 128:(f + 1) * 128], ps)
                                nc.sync.dma_start(
                                    y_d.ap()[t * 128:(t + 1) * 128, :],
                                    o_tm)


def _get_nc():
    global _NC_CACHE
    if _NC_CACHE is None:
        _NC_CACHE = _build()
    return _NC_CACHE


_WEIGHT_NAMES = ["w_qkv", "b_qkv", "w_out", "b_out", "w_c1", "b_c1",
                 "w_c2", "b_c2", "ln_w", "ln_b"]


def run(inputs, trace=False):
    nc = _get_nc()
    xs = np.ascontiguousarray(np.asarray(inputs["x"], dtype=np.float32))
    assert xs.shape == (B, T, C), xs.shape
    shared = {k: np.ascontiguousarray(np.asarray(inputs[k], np.float32))
              for k in _WEIGHT_NAMES}
    in_maps = [dict(shared, x=xs[c]) for c in range(B)]
    res = bass_utils.run_bass_kernel_spmd(
        nc, in_maps, core_ids=list(range(B)), trace=trace)
    out = np.stack([r["y"] for r in res.results], axis=0)
    return out, res


def kernel(**inputs):
    out, _ = run(inputs, trace=False)
    return out

